# revision 23
# baseline (speedup 1.0000x reference)
"""Trainium2 Bass kernel for nn_CoocOpModel.

out[b,k] = sum_ij func[b,i] * C[i,j,k] * arg[b,j] + op_mask[b,k]

with B=64, V=8192, C: [V, V, 3] f32 (805 MB -> memory-bound).

Distribution: C is sharded along its first (i) axis across 8 NeuronCores
(1024 rows / ~100 MB per core); func/arg are replicated. Each core computes
the partial contraction over its local i range:

  T[b, (j,k)] = sum_i f[b,i] * C[i, (j,k)]      (TensorE, f chunks stationary,
                                                 PSUM accumulation over i-chunks)
  partial[b,k] = sum_j T[b,j,k] * a[b,j]        (DVE tensor_tensor_reduce,
                                                 strided PSUM read per k)

plus op_mask/8, then an AllReduce over the 8 cores sums the partials (and the
8 mask/8 contributions) into the full [64, 3] result on every core.
"""

import numpy as np

B = 64
V = 8192
K3 = 3
N_CORES = 8


def _build_nc(v_local, j_total, b, n_cores):
    import concourse.bass as bass
    import concourse.mybir as mybir
    import concourse.tile as tile
    from concourse import bacc

    f32 = mybir.dt.float32
    f16 = mybir.dt.float16
    P = 128
    IC = v_local // P            # i-chunks per core
    NK = j_total * K3            # total moving columns
    J_T = 128                    # j per psum tile
    NT = J_T * K3                # psum tile cols (384)
    TPC = 4                      # psum tiles per DMA chunk
    JCHUNK = J_T * TPC           # 512 j per chunk
    NCHUNK = NT * TPC            # 1536 cols per chunk
    NCHUNKS = j_total // JCHUNK
    NTILES = NCHUNKS * TPC
    assert v_local % P == 0 and j_total % JCHUNK == 0

    nc = bacc.Bacc(None, target_bir_lowering=False, debug=False,
                   num_devices=n_cores, num_swdge_queues=2)
    f_t = nc.declare_dram_parameter("f_t", [v_local, b], f32, isOutput=False)
    arg_v = nc.declare_dram_parameter("arg_v", [b, j_total], f32,
                                      isOutput=False)
    flags = nc.declare_dram_parameter("flags", [b, 2], f32, isOutput=False)
    cooc = nc.declare_dram_parameter("cooc", [v_local, NK], f32,
                                     isOutput=False)
    out = nc.declare_dram_parameter("out", [b, K3], f32, isOutput=True)

    with tile.TileContext(nc) as tc, \
            tc.tile_pool(name="pers", bufs=1) as pers, \
            tc.tile_pool(name="cchunk", bufs=10) as cpool, \
            tc.tile_pool(name="psum", bufs=2 * TPC, space="PSUM") as ppool, \
            tc.tile_pool(name="scr", bufs=2) as spool, \
            tc.tile_pool(name="dram", bufs=1, space="DRAM") as dpool:

        # ---- persistent inputs -------------------------------------------
        # C window 0 is issued first (see below) so the PE's first matmul
        # isn't queued behind these loads.
        # f^T in fp16 (i on partitions); SWDGE casts f32->fp16 during DMA.
        # fp32 matmuls run as two PE passes on TRN2 -- fp16 inputs halve the
        # PE work while PSUM accumulation stays fp32. fp16 (10-bit mantissa)
        # keeps elementwise error ~3e-3; C~N(0,1), f in [0,1) fit the range.
        fsb = pers.tile([P, IC * b], f16)
        asb = pers.tile([b, j_total], f32)           # arg rows, b on partitions
        flg = pers.tile([b, 2], f32)

        def load_persistent():
            nc.gpsimd.dma_start(
                out=fsb[:].rearrange("p (c b) -> p c b", c=IC),
                in_=f_t[:].rearrange("(c p) b -> p c b", p=P),
            )
            for q in range(8):
                qs = j_total // 8
                nc.sync.dma_start(out=asb[:, q * qs:(q + 1) * qs],
                                  in_=arg_v[:, q * qs:(q + 1) * qs])
            nc.sync.dma_start(out=flg[:], in_=flags[:])

        slots = pers.tile([b, NTILES * K3], f32)     # per-psum-tile partials
        mask8 = pers.tile([b, K3], f32)              # op_mask / n_cores
        ms = pers.tile([b, 1], f32)

        # ---- main streaming loop over C ----------------------------------
        # C shard viewed as [p=128, ic, n]: row i = ic*128 + p.
        # DMA granularity: one transfer per (window cc, ic-pair) moving
        # [128, 2, NWIN] with NWIN-column (12 KB f32) contiguous runs.
        cooc_r = cooc[:].rearrange("(c p) n -> p c n", p=P)
        NWIN = min(4 * NCHUNK, NK)             # 6144 cols = 2048 j per window
        NWINS = NK // NWIN
        HALVES = NWIN // NCHUNK

        for cc in range(NWINS):
            ic_tiles = []
            for ic in range(IC):
                ct = cpool.tile([P, NWIN], f16, tag="cpair",
                                name=f"cp_{cc}_{ic}")
                nc.gpsimd.dma_start(
                    out=ct[:],
                    in_=cooc_r[:, ic, cc * NWIN:(cc + 1) * NWIN],
                )
                ic_tiles.append(ct)
                if cc == 0 and ic == 0:
                    load_persistent()
            for half in range(HALVES):
                ptiles = [ppool.tile([b, NT], f32, tag="pt",
                                     name=f"pt_{cc}_{half}_{t}")
                          for t in range(TPC)]
                for ic in range(IC):
                    ct = ic_tiles[ic]
                    base = half * NCHUNK
                    for t in range(TPC):
                        nc.tensor.matmul(
                            out=ptiles[t][:],
                            lhsT=fsb[:, ic * b:(ic + 1) * b],
                            rhs=ct[:, base + t * NT:base + (t + 1) * NT],
                            start=(ic == 0),
                            stop=(ic == IC - 1),
                        )
                for t in range(TPC):
                    g = (cc * HALVES + half) * TPC + t
                    j0 = g * J_T
                    # prod[b,(j,k)] = T[b,(j,k)] * a[b,j] (stride-0 k bcast)
                    scr = spool.tile([b, NT], f32, tag="scr", name=f"scr_{g}")
                    a_sl = asb[:, j0:j0 + J_T]
                    a_bc = bass.AP(a_sl.tensor, a_sl.offset,
                                   [list(a_sl.ap[0]), list(a_sl.ap[1]),
                                    [0, K3]])
                    nc.vector.tensor_mul(
                        scr[:].rearrange("p (j k) -> p j k", k=K3),
                        ptiles[t][:].rearrange("p (j k) -> p j k", k=K3),
                        a_bc,
                    )
                    # slots[b, g, k] = sum_j prod[b, j, k]
                    nc.vector.tensor_reduce(
                        out=slots[:, g * K3:(g + 1) * K3],
                        in_=scr[:].rearrange("p (j k) -> p k j", k=K3),
                        axis=mybir.AxisListType.X,
                        op=mybir.AluOpType.add,
                    )

        # ---- fold partials + mask, AllReduce, store ----------------------
        racc = pers.tile([b, K3], f32)

        # op_mask / n_cores from the two flag columns:
        # col0 = q*a1 ; col1 = q*(a1 + a2 - a1*a2) ; col2 = 0,  q = -1e9/ncores
        q = -1.0e9 / n_cores
        nc.vector.tensor_mul(ms[:], flg[:, 0:1], flg[:, 1:2])
        nc.vector.tensor_add(mask8[:, 1:2], flg[:, 0:1], flg[:, 1:2])
        nc.vector.tensor_sub(mask8[:, 1:2], mask8[:, 1:2], ms[:])
        nc.vector.tensor_scalar_mul(mask8[:, 1:2], mask8[:, 1:2], q)
        nc.vector.tensor_scalar_mul(mask8[:, 0:1], flg[:, 0:1], q)
        nc.vector.memset(mask8[:, 2:3], 0.0)
        nc.vector.tensor_reduce(
            out=racc[:],
            in_=slots[:].rearrange("p (g k) -> p k g", k=K3),
            axis=mybir.AxisListType.X,
            op=mybir.AluOpType.add,
        )
        nc.vector.tensor_add(racc[:], racc[:], mask8[:])

        bounce_in = dpool.tile([b, K3], f32, tag="bin")
        bounce_out = dpool.tile([b, K3], f32, tag="bout")
        nc.sync.dma_start(out=bounce_in[:], in_=racc[:])
        nc.gpsimd.collective_compute(
            "AllReduce",
            mybir.AluOpType.add,
            replica_groups=[list(range(n_cores))],
            ins=[bounce_in.opt()],
            outs=[bounce_out.opt()],
        )
        nc.sync.dma_start(out=out[:], in_=bounce_out[:])

    nc.compile()
    return nc


_NC_CACHE = {}


def _get_nc(v_local, j_total, b, n_cores):
    key = (v_local, j_total, b, n_cores)
    if key not in _NC_CACHE:
        _NC_CACHE[key] = _build_nc(v_local, j_total, b, n_cores)
    return _NC_CACHE[key]


def make_in_maps(func, arg, cooccurrences, n_cores):
    """Shard the full inputs for SPMD execution (host-side layout only)."""
    func = np.ascontiguousarray(np.asarray(func, dtype=np.float32))
    arg = np.ascontiguousarray(np.asarray(arg, dtype=np.float32))
    cooc = np.asarray(cooccurrences, dtype=np.float32)
    v = cooc.shape[0]
    v_local = v // n_cores
    f_t = np.ascontiguousarray(func[:, :v].T)            # [V, B]
    arg_v = np.ascontiguousarray(arg[:, :v])             # [B, V]
    flags = np.ascontiguousarray(func[:, v:v + 2])       # [B, 2]
    in_maps = []
    for c in range(n_cores):
        sl = slice(c * v_local, (c + 1) * v_local)
        in_maps.append({
            "f_t": np.ascontiguousarray(f_t[sl]),
            "arg_v": arg_v,
            "flags": flags,
            "cooc": cooc[sl].reshape(v_local, v * K3),
        })
    return in_maps


def kernel(func, arg, cooccurrences):
    from concourse.bass_utils import run_bass_kernel_spmd

    in_maps = make_in_maps(func, arg, cooccurrences, N_CORES)
    nc = _get_nc(V // N_CORES, V, B, N_CORES)
    res = run_bass_kernel_spmd(nc, in_maps, core_ids=list(range(N_CORES)))
    return np.asarray(res.results[0]["out"], dtype=np.float32)


# revision 24
# speedup vs baseline: 1.0146x; 1.0146x over previous
"""Trainium2 Bass kernel for nn_CoocOpModel.

out[b,k] = sum_ij func[b,i] * C[i,j,k] * arg[b,j] + op_mask[b,k]

with B=64, V=8192, C: [V, V, 3] f32 (805 MB -> memory-bound).

Distribution: C is sharded along its first (i) axis across 8 NeuronCores
(1024 rows / ~100 MB per core); func/arg are replicated. Each core computes
the partial contraction over its local i range:

  T[b, (j,k)] = sum_i f[b,i] * C[i, (j,k)]      (TensorE, f chunks stationary,
                                                 PSUM accumulation over i-chunks)
  partial[b,k] = sum_j T[b,j,k] * a[b,j]        (DVE tensor_tensor_reduce,
                                                 strided PSUM read per k)

plus op_mask/8, then an AllReduce over the 8 cores sums the partials (and the
8 mask/8 contributions) into the full [64, 3] result on every core.
"""

import numpy as np

B = 64
V = 8192
K3 = 3
N_CORES = 8


def _build_nc(v_local, j_total, b, n_cores):
    import concourse.bass as bass
    import concourse.mybir as mybir
    import concourse.tile as tile
    from concourse import bacc

    f32 = mybir.dt.float32
    f16 = mybir.dt.float16
    P = 128
    IC = v_local // P            # i-chunks per core
    NK = j_total * K3            # total moving columns
    J_T = 128                    # j per psum tile
    NT = J_T * K3                # psum tile cols (384)
    TPC = 4                      # psum tiles per DMA chunk
    JCHUNK = J_T * TPC           # 512 j per chunk
    NCHUNK = NT * TPC            # 1536 cols per chunk
    NCHUNKS = j_total // JCHUNK
    NTILES = NCHUNKS * TPC
    assert v_local % P == 0 and j_total % JCHUNK == 0

    nc = bacc.Bacc(None, target_bir_lowering=False, debug=False,
                   num_devices=n_cores, num_swdge_queues=2)
    f_t = nc.declare_dram_parameter("f_t", [v_local, b], f32, isOutput=False)
    arg_v = nc.declare_dram_parameter("arg_v", [b, j_total], f32,
                                      isOutput=False)
    flags = nc.declare_dram_parameter("flags", [b, 2], f32, isOutput=False)
    cooc = nc.declare_dram_parameter("cooc", [v_local, NK], f32,
                                     isOutput=False)
    out = nc.declare_dram_parameter("out", [b, K3], f32, isOutput=True)

    with tile.TileContext(nc) as tc, \
            tc.tile_pool(name="pers", bufs=1) as pers, \
            tc.tile_pool(name="cchunk", bufs=10) as cpool, \
            tc.tile_pool(name="psum", bufs=2 * TPC, space="PSUM") as ppool, \
            tc.tile_pool(name="scr", bufs=2) as spool, \
            tc.tile_pool(name="dram", bufs=1, space="DRAM") as dpool:

        # ---- persistent inputs -------------------------------------------
        # C window 0 is issued first (see below) so the PE's first matmul
        # isn't queued behind these loads.
        # f^T in fp16 (i on partitions); SWDGE casts f32->fp16 during DMA.
        # fp32 matmuls run as two PE passes on TRN2 -- fp16 inputs halve the
        # PE work while PSUM accumulation stays fp32. fp16 (10-bit mantissa)
        # keeps elementwise error ~3e-3; C~N(0,1), f in [0,1) fit the range.
        fsb = pers.tile([P, IC * b], f16)
        asb = pers.tile([b, j_total], f32)           # arg rows, b on partitions
        flg = pers.tile([b, 2], f32)
        nc.gpsimd.dma_start(
            out=fsb[:].rearrange("p (c b) -> p c b", c=IC),
            in_=f_t[:].rearrange("(c p) b -> p c b", p=P),
        )
        for q in range(8):
            qs = j_total // 8
            nc.sync.dma_start(out=asb[:, q * qs:(q + 1) * qs],
                              in_=arg_v[:, q * qs:(q + 1) * qs])
        nc.sync.dma_start(out=flg[:], in_=flags[:])

        slots = pers.tile([b, NTILES * K3], f32)     # per-psum-tile partials
        mask8 = pers.tile([b, K3], f32)              # op_mask / n_cores
        ms = pers.tile([b, 1], f32)

        # ---- main streaming loop over C ----------------------------------
        # C shard viewed as [p=128, ic, n]: row i = ic*128 + p.
        # DMA granularity: one transfer per (window cc, ic-pair) moving
        # [128, 2, NWIN] with NWIN-column (12 KB f32) contiguous runs.
        cooc_r = cooc[:].rearrange("(c p) n -> p c n", p=P)
        NWIN = min(4 * NCHUNK, NK)             # 6144 cols = 2048 j per window
        NWINS = NK // NWIN
        HALVES = NWIN // NCHUNK

        for cc in range(NWINS):
            ic_tiles = []
            for ic in range(IC):
                ct = cpool.tile([P, NWIN], f16, tag="cpair",
                                name=f"cp_{cc}_{ic}")
                nc.gpsimd.dma_start(
                    out=ct[:],
                    in_=cooc_r[:, ic, cc * NWIN:(cc + 1) * NWIN],
                )
                ic_tiles.append(ct)
            for half in range(HALVES):
                ptiles = [ppool.tile([b, NT], f32, tag="pt",
                                     name=f"pt_{cc}_{half}_{t}")
                          for t in range(TPC)]
                for ic in range(IC):
                    ct = ic_tiles[ic]
                    base = half * NCHUNK
                    for t in range(TPC):
                        nc.tensor.matmul(
                            out=ptiles[t][:],
                            lhsT=fsb[:, ic * b:(ic + 1) * b],
                            rhs=ct[:, base + t * NT:base + (t + 1) * NT],
                            start=(ic == 0),
                            stop=(ic == IC - 1),
                        )
                for t in range(TPC):
                    g = (cc * HALVES + half) * TPC + t
                    j0 = g * J_T
                    # prod[b,(j,k)] = T[b,(j,k)] * a[b,j] (stride-0 k bcast)
                    scr = spool.tile([b, NT], f32, tag="scr", name=f"scr_{g}")
                    a_sl = asb[:, j0:j0 + J_T]
                    a_bc = bass.AP(a_sl.tensor, a_sl.offset,
                                   [list(a_sl.ap[0]), list(a_sl.ap[1]),
                                    [0, K3]])
                    nc.vector.tensor_mul(
                        scr[:].rearrange("p (j k) -> p j k", k=K3),
                        ptiles[t][:].rearrange("p (j k) -> p j k", k=K3),
                        a_bc,
                    )
                    # slots[b, g, k] = sum_j prod[b, j, k]
                    nc.vector.tensor_reduce(
                        out=slots[:, g * K3:(g + 1) * K3],
                        in_=scr[:].rearrange("p (j k) -> p k j", k=K3),
                        axis=mybir.AxisListType.X,
                        op=mybir.AluOpType.add,
                    )

        # ---- fold partials + mask, AllReduce, store ----------------------
        racc = pers.tile([b, K3], f32)

        # op_mask / n_cores from the two flag columns:
        # col0 = q*a1 ; col1 = q*(a1 + a2 - a1*a2) ; col2 = 0,  q = -1e9/ncores
        q = -1.0e9 / n_cores
        nc.vector.tensor_mul(ms[:], flg[:, 0:1], flg[:, 1:2])
        nc.vector.tensor_add(mask8[:, 1:2], flg[:, 0:1], flg[:, 1:2])
        nc.vector.tensor_sub(mask8[:, 1:2], mask8[:, 1:2], ms[:])
        nc.vector.tensor_scalar_mul(mask8[:, 1:2], mask8[:, 1:2], q)
        nc.vector.tensor_scalar_mul(mask8[:, 0:1], flg[:, 0:1], q)
        nc.vector.memset(mask8[:, 2:3], 0.0)
        nc.vector.tensor_reduce(
            out=racc[:],
            in_=slots[:].rearrange("p (g k) -> p k g", k=K3),
            axis=mybir.AxisListType.X,
            op=mybir.AluOpType.add,
        )
        nc.vector.tensor_add(racc[:], racc[:], mask8[:])

        bounce_in = dpool.tile([b, K3], f32, tag="bin")
        bounce_out = dpool.tile([b, K3], f32, tag="bout")
        nc.sync.dma_start(out=bounce_in[:], in_=racc[:])
        nc.gpsimd.collective_compute(
            "AllReduce",
            mybir.AluOpType.add,
            replica_groups=[list(range(n_cores))],
            ins=[bounce_in.opt()],
            outs=[bounce_out.opt()],
        )
        nc.sync.dma_start(out=out[:], in_=bounce_out[:])

    nc.compile()
    return nc


_NC_CACHE = {}


def _get_nc(v_local, j_total, b, n_cores):
    key = (v_local, j_total, b, n_cores)
    if key not in _NC_CACHE:
        _NC_CACHE[key] = _build_nc(v_local, j_total, b, n_cores)
    return _NC_CACHE[key]


def make_in_maps(func, arg, cooccurrences, n_cores):
    """Shard the full inputs for SPMD execution (host-side layout only)."""
    func = np.ascontiguousarray(np.asarray(func, dtype=np.float32))
    arg = np.ascontiguousarray(np.asarray(arg, dtype=np.float32))
    cooc = np.asarray(cooccurrences, dtype=np.float32)
    v = cooc.shape[0]
    v_local = v // n_cores
    f_t = np.ascontiguousarray(func[:, :v].T)            # [V, B]
    arg_v = np.ascontiguousarray(arg[:, :v])             # [B, V]
    flags = np.ascontiguousarray(func[:, v:v + 2])       # [B, 2]
    in_maps = []
    for c in range(n_cores):
        sl = slice(c * v_local, (c + 1) * v_local)
        in_maps.append({
            "f_t": np.ascontiguousarray(f_t[sl]),
            "arg_v": arg_v,
            "flags": flags,
            "cooc": cooc[sl].reshape(v_local, v * K3),
        })
    return in_maps


def kernel(func, arg, cooccurrences):
    from concourse.bass_utils import run_bass_kernel_spmd

    in_maps = make_in_maps(func, arg, cooccurrences, N_CORES)
    nc = _get_nc(V // N_CORES, V, B, N_CORES)
    res = run_bass_kernel_spmd(nc, in_maps, core_ids=list(range(N_CORES)))
    return np.asarray(res.results[0]["out"], dtype=np.float32)


# revision 25
# speedup vs baseline: 1.0147x; 1.0001x over previous
"""Trainium2 Bass kernel for nn_CoocOpModel.

out[b,k] = sum_ij func[b,i] * C[i,j,k] * arg[b,j] + op_mask[b,k]

with B=64, V=8192, C: [V, V, 3] f32 (805 MB -> memory-bound).

Distribution: C is sharded along its first (i) axis across 8 NeuronCores
(1024 rows / ~100 MB per core); func/arg are replicated. Each core computes
the partial contraction over its local i range:

  T[b, (j,k)] = sum_i f[b,i] * C[i, (j,k)]      (TensorE, f chunks stationary,
                                                 PSUM accumulation over i-chunks)
  partial[b,k] = sum_j T[b,j,k] * a[b,j]        (DVE tensor_tensor_reduce,
                                                 strided PSUM read per k)

plus op_mask/8, then an AllReduce over the 8 cores sums the partials (and the
8 mask/8 contributions) into the full [64, 3] result on every core.
"""

import numpy as np

B = 64
V = 8192
K3 = 3
N_CORES = 8


def _build_nc(v_local, j_total, b, n_cores):
    import concourse.bass as bass
    import concourse.mybir as mybir
    import concourse.tile as tile
    from concourse import bacc

    f32 = mybir.dt.float32
    f16 = mybir.dt.float16
    P = 128
    IC = v_local // P            # i-chunks per core
    NK = j_total * K3            # total moving columns
    J_T = 128                    # j per psum tile
    NT = J_T * K3                # psum tile cols (384)
    TPC = 4                      # psum tiles per DMA chunk
    JCHUNK = J_T * TPC           # 512 j per chunk
    NCHUNK = NT * TPC            # 1536 cols per chunk
    NCHUNKS = j_total // JCHUNK
    NTILES = NCHUNKS * TPC
    assert v_local % P == 0 and j_total % JCHUNK == 0

    nc = bacc.Bacc(None, target_bir_lowering=False, debug=False,
                   num_devices=n_cores, num_swdge_queues=2)
    f_t = nc.declare_dram_parameter("f_t", [v_local, b], f32, isOutput=False)
    arg_v = nc.declare_dram_parameter("arg_v", [b, j_total], f32,
                                      isOutput=False)
    flags = nc.declare_dram_parameter("flags", [b, 2], f32, isOutput=False)
    cooc = nc.declare_dram_parameter("cooc", [v_local, NK], f32,
                                     isOutput=False)
    out = nc.declare_dram_parameter("out", [b, K3], f32, isOutput=True)

    with tile.TileContext(nc) as tc, \
            tc.tile_pool(name="pers", bufs=1) as pers, \
            tc.tile_pool(name="cchunk", bufs=10) as cpool, \
            tc.tile_pool(name="psum", bufs=2 * TPC, space="PSUM") as ppool, \
            tc.tile_pool(name="scr", bufs=2) as spool, \
            tc.tile_pool(name="dram", bufs=1, space="DRAM") as dpool:

        # ---- persistent inputs -------------------------------------------
        # C window 0 is issued first (see below) so the PE's first matmul
        # isn't queued behind these loads.
        # f^T in fp16 (i on partitions); SWDGE casts f32->fp16 during DMA.
        # fp32 matmuls run as two PE passes on TRN2 -- fp16 inputs halve the
        # PE work while PSUM accumulation stays fp32. fp16 (10-bit mantissa)
        # keeps elementwise error ~3e-3; C~N(0,1), f in [0,1) fit the range.
        fsb = pers.tile([P, IC * b], f16)
        asb = pers.tile([b, j_total], f32)           # arg rows, b on partitions
        flg = pers.tile([b, 2], f32)
        nc.gpsimd.dma_start(
            out=fsb[:].rearrange("p (c b) -> p c b", c=IC),
            in_=f_t[:].rearrange("(c p) b -> p c b", p=P),
        )
        for q in range(8):
            qs = j_total // 8
            nc.sync.dma_start(out=asb[:, q * qs:(q + 1) * qs],
                              in_=arg_v[:, q * qs:(q + 1) * qs])
        nc.sync.dma_start(out=flg[:], in_=flags[:])

        slots = pers.tile([b, NTILES * K3], f32)     # per-psum-tile partials
        mask8 = pers.tile([b, K3], f32)              # op_mask / n_cores
        ms = pers.tile([b, 1], f32)

        # op_mask / n_cores from the two flag columns:
        # col0 = q*a1 ; col1 = q*(a1 + a2 - a1*a2) ; col2 = 0,  q = -1e9/ncores
        q = -1.0e9 / n_cores
        nc.vector.tensor_mul(ms[:], flg[:, 0:1], flg[:, 1:2])
        nc.vector.tensor_add(mask8[:, 1:2], flg[:, 0:1], flg[:, 1:2])
        nc.vector.tensor_sub(mask8[:, 1:2], mask8[:, 1:2], ms[:])
        nc.vector.tensor_scalar_mul(mask8[:, 1:2], mask8[:, 1:2], q)
        nc.vector.tensor_scalar_mul(mask8[:, 0:1], flg[:, 0:1], q)
        nc.vector.memset(mask8[:, 2:3], 0.0)

        # ---- main streaming loop over C ----------------------------------
        # C shard viewed as [p=128, ic, n]: row i = ic*128 + p.
        # DMA granularity: one transfer per (window cc, ic-pair) moving
        # [128, 2, NWIN] with NWIN-column (12 KB f32) contiguous runs.
        cooc_r = cooc[:].rearrange("(c p) n -> p c n", p=P)
        NWIN = min(4 * NCHUNK, NK)             # 6144 cols = 2048 j per window
        NWINS = NK // NWIN
        HALVES = NWIN // NCHUNK

        for cc in range(NWINS):
            ic_tiles = []
            for ic in range(IC):
                ct = cpool.tile([P, NWIN], f16, tag="cpair",
                                name=f"cp_{cc}_{ic}")
                nc.gpsimd.dma_start(
                    out=ct[:],
                    in_=cooc_r[:, ic, cc * NWIN:(cc + 1) * NWIN],
                )
                ic_tiles.append(ct)
            for half in range(HALVES):
                ptiles = [ppool.tile([b, NT], f32, tag="pt",
                                     name=f"pt_{cc}_{half}_{t}")
                          for t in range(TPC)]
                for ic in range(IC):
                    ct = ic_tiles[ic]
                    base = half * NCHUNK
                    for t in range(TPC):
                        nc.tensor.matmul(
                            out=ptiles[t][:],
                            lhsT=fsb[:, ic * b:(ic + 1) * b],
                            rhs=ct[:, base + t * NT:base + (t + 1) * NT],
                            start=(ic == 0),
                            stop=(ic == IC - 1),
                        )
                for t in range(TPC):
                    g = (cc * HALVES + half) * TPC + t
                    j0 = g * J_T
                    # prod[b,(j,k)] = T[b,(j,k)] * a[b,j] (stride-0 k bcast)
                    scr = spool.tile([b, NT], f32, tag="scr", name=f"scr_{g}")
                    a_sl = asb[:, j0:j0 + J_T]
                    a_bc = bass.AP(a_sl.tensor, a_sl.offset,
                                   [list(a_sl.ap[0]), list(a_sl.ap[1]),
                                    [0, K3]])
                    nc.vector.tensor_mul(
                        scr[:].rearrange("p (j k) -> p j k", k=K3),
                        ptiles[t][:].rearrange("p (j k) -> p j k", k=K3),
                        a_bc,
                    )
                    # slots[b, g, k] = sum_j prod[b, j, k]
                    nc.vector.tensor_reduce(
                        out=slots[:, g * K3:(g + 1) * K3],
                        in_=scr[:].rearrange("p (j k) -> p k j", k=K3),
                        axis=mybir.AxisListType.X,
                        op=mybir.AluOpType.add,
                    )

        # ---- fold partials + mask, AllReduce, store ----------------------
        racc = pers.tile([b, K3], f32)
        nc.vector.tensor_reduce(
            out=racc[:],
            in_=slots[:].rearrange("p (g k) -> p k g", k=K3),
            axis=mybir.AxisListType.X,
            op=mybir.AluOpType.add,
        )
        nc.vector.tensor_add(racc[:], racc[:], mask8[:])

        bounce_in = dpool.tile([b, K3], f32, tag="bin")
        bounce_out = dpool.tile([b, K3], f32, tag="bout")
        nc.sync.dma_start(out=bounce_in[:], in_=racc[:])
        nc.gpsimd.collective_compute(
            "AllReduce",
            mybir.AluOpType.add,
            replica_groups=[list(range(n_cores))],
            ins=[bounce_in.opt()],
            outs=[bounce_out.opt()],
        )
        nc.sync.dma_start(out=out[:], in_=bounce_out[:])

    nc.compile()
    return nc


_NC_CACHE = {}


def _get_nc(v_local, j_total, b, n_cores):
    key = (v_local, j_total, b, n_cores)
    if key not in _NC_CACHE:
        _NC_CACHE[key] = _build_nc(v_local, j_total, b, n_cores)
    return _NC_CACHE[key]


def make_in_maps(func, arg, cooccurrences, n_cores):
    """Shard the full inputs for SPMD execution (host-side layout only)."""
    func = np.ascontiguousarray(np.asarray(func, dtype=np.float32))
    arg = np.ascontiguousarray(np.asarray(arg, dtype=np.float32))
    cooc = np.asarray(cooccurrences, dtype=np.float32)
    v = cooc.shape[0]
    v_local = v // n_cores
    f_t = np.ascontiguousarray(func[:, :v].T)            # [V, B]
    arg_v = np.ascontiguousarray(arg[:, :v])             # [B, V]
    flags = np.ascontiguousarray(func[:, v:v + 2])       # [B, 2]
    in_maps = []
    for c in range(n_cores):
        sl = slice(c * v_local, (c + 1) * v_local)
        in_maps.append({
            "f_t": np.ascontiguousarray(f_t[sl]),
            "arg_v": arg_v,
            "flags": flags,
            "cooc": cooc[sl].reshape(v_local, v * K3),
        })
    return in_maps


def kernel(func, arg, cooccurrences):
    from concourse.bass_utils import run_bass_kernel_spmd

    in_maps = make_in_maps(func, arg, cooccurrences, N_CORES)
    nc = _get_nc(V // N_CORES, V, B, N_CORES)
    res = run_bass_kernel_spmd(nc, in_maps, core_ids=list(range(N_CORES)))
    return np.asarray(res.results[0]["out"], dtype=np.float32)
